# revision 6
# baseline (speedup 1.0000x reference)
"""Causal self-attention (RoPE) Trainium2 kernel, v2.

Model: B=2, T=2048, D=2048, 16 heads x 128 head-dim, RoPE theta=1e4.

Sharding (8 cores): cores 0-3 own batch 0, cores 4-7 own batch 1; within a
batch group each core owns 4 heads (tensor parallel over heads for QKV /
attention, row-parallel over w_out). Host sums the 4 partial outputs per
batch.

Key HW-calibrated choices (this axon-trn2 environment):
 - All matmuls fp32r: 217ns per [128x128]x[128,512] (bf16 is 420ns, fp16
   324ns, fp8 441ns, f32 806ns). All tensors live in SBUF as f32 and are
   bitcast to f32r at the matmul.
 - Q/K are produced TRANSPOSED directly ([head_dim, t] layout) by making
   the weight tile the stationary operand, so no PE transposes are needed
   anywhere. RoPE's rotate-half becomes a +-1 permutation matmul on the PE;
   the cos/sin elementwise work is 3 fused scalar_tensor_tensor DVE ops
   (~0.4-0.5us each) per (tensor, head, 512-token chunk).
 - The causal mask is applied ON THE PE: one extra matmul accumulating
   step-matrix^T @ shifted-delta-matrix into the logits PSUM (the DVE adds
   it replaced cost ~1.6us each).
 - exp runs on ACT straight from PSUM (259ns per [128,512]); PSUM->SBUF
   staging uses ACT Identity (314ns; Copy costs 892ns).
 - DVE tensor_tensor is slow here (~1.6us per [128,512]); every DVE binary
   op uses the fused scalar_tensor_tensor form instead (~0.4us).
 - DMA: ~274 GB/s/core reads, ~435 GB/s writes under full 8-core SPMD.
   x^T is streamed three times (q-pass, k-pass, v-pass) to keep SBUF under
   budget; every blob region is partition-major so each dma_start has one
   contiguous line per partition.

Attention uses the S^T layout: ST[k,q] = (K^T)^T Q^T so probabilities
leave the exp already transposed for the AV matmul. Softmax denominators
come from a ones-row matmul; max-subtraction is skipped (logits are O(5)
here - exp cannot overflow; verified on the actual inputs).
"""

import sys

sys.path.insert(0, "/opt/trn_rl_repo")

import numpy as np

import concourse.bass as bass
import concourse.mybir as mybir
from concourse import tile
from concourse.bass_utils import run_bass_kernel_spmd

F32 = mybir.dt.float32
F32R = mybir.dt.float32r
BF16 = mybir.dt.bfloat16
AF = mybir.ActivationFunctionType
ALU = mybir.AluOpType

B, T, D = 2, 2048, 2048
H, HD = 16, 128
N_CORES = 8
GROUPS = 2                   # batch groups
CPG = N_CORES // GROUPS      # cores per group (4)
HPC = H // CPG               # heads per core (4)
DL = HPC * HD                # local head dims (512)
ROPE_THETA = 10000.0
SCALE = float(HD) ** -0.5
NEG = -1.0e6                 # additive mask; exp(NEG*SCALE) == 0

KI_N = D // 128              # 16 contraction tiles over D
CH_N = T // 512              # 4 token chunks of 512
TPB = T // 128               # 16 t-tiles
QC_N = T // 512              # 4 q-chunks of 512
NC_N = D // 512              # 4 n-chunks for the output projection

# ---- blob layout (f32 elements) ----
_off = 0
def _reg(n):
    global _off
    o = _off
    _off += n
    return o

X4_OFF = _reg(CH_N * 128 * KI_N * 512)       # [chunk, p, ki, 512t]
WQ_OFF = _reg(128 * HPC * KI_N * 128)        # [p, h, ki, 128j]
WK_OFF = _reg(128 * HPC * KI_N * 128)
WV_OFF = _reg(128 * KI_N * 512)              # [p, ki, 512c]
WO_OFF = _reg(128 * HPC * D)                 # [p, h, 2048n]
COS_OFF = _reg(128 * T)                      # [p, t] cos(t*invf[p%64])
SIN_OFF = _reg(128 * T)                      # [p, t] sin (unsigned)
MSK_OFF = _reg(128 * 4 * 512)                # [j, r, 512qf] shifted deltas
WSTEP_OFF = _reg(128 * 128)                  # [j, kp] = 1 if j<=kp else 0
PROT_OFF = _reg(128 * 128)                   # rotate-half permutation lhsT
BLOB_N = _off


def _split_multi_waits(nc):
    """This container's walrus accepts at most ONE semaphore wait per
    instruction; hoist extra waits onto single-wait NoOps inserted right
    before the instruction on the same engine (sequencers run in order, so
    semantics are unchanged)."""
    n = 0
    for f in nc.m.functions:
        for b in f.blocks:
            il = b.instructions
            if not any(
                i.sync_info is not None and len(i.sync_info.on_wait) > 1
                for i in il
            ):
                continue
            out = []
            for inst in il:
                si = inst.sync_info
                if si is not None and len(si.on_wait) > 1:
                    waits = list(si.on_wait)
                    for w in waits[:-1]:
                        nop = mybir.InstNoOp(
                            name=nc.get_next_instruction_name(), ins=[], outs=[]
                        )
                        nop.engine = inst.engine
                        nop.sync_info = mybir.SyncInfo(on_wait=[w], on_update=[])
                        nc.register_instruction(nop)
                        out.append(nop)
                        n += 1
                    inst.sync_info = mybir.SyncInfo(
                        on_wait=[waits[-1]], on_update=list(si.on_update)
                    )
                out.append(inst)
            il[:] = out
    return n


def _emit_body(nc, tc, io, stk):
    blob = io["blob"]
    y = io["y"]

    persist = stk.enter_context(tc.tile_pool(name="persist", bufs=1))
    # qT/kT: [128 head_dim, head, t] f32 (bitcast to f32r at matmuls)
    qT = persist.tile([128, HPC, T], F32R, name="qT")
    kT = persist.tile([128, HPC, T], F32R, name="kT")
    prot = persist.tile([128, 128], F32R, name="prot")
    nc.sync.dma_start(
        prot[:], blob[PROT_OFF:PROT_OFF + 128 * 128].rearrange(
            "(p j) -> p j", p=128
        ),
    )

    def qk_pass(which, dst):
        """One pass producing dst = RoPE(tensor) in [hd, h, t] layout."""
        woff = WQ_OFF if which == 0 else WK_OFF
        with (
            tc.tile_pool(name=f"w{which}", bufs=1) as wp,
            tc.tile_pool(name=f"x{which}", bufs=2) as xp,
            tc.tile_pool(name=f"s{which}", bufs=2) as sp,
            tc.tile_pool(name=f"ps{which}", bufs=2, space="PSUM") as psp,
            tc.tile_pool(name=f"pr{which}", bufs=2, space="PSUM") as prp,
        ):
            wt = wp.tile([128, HPC, KI_N, 128], F32R, name="wt")
            nc.sync.dma_start(
                wt[:],
                blob[woff:woff + 128 * HPC * KI_N * 128].rearrange(
                    "(p h k j) -> p h k j", p=128, h=HPC, k=KI_N
                ),
            )
            for c in range(CH_N):
                xc = xp.tile([128, KI_N, 512], F32R, name="xc")
                xoff = X4_OFF + c * 128 * KI_N * 512
                nc.sync.dma_start(
                    xc[:],
                    blob[xoff:xoff + 128 * KI_N * 512].rearrange(
                        "(p k t) -> p k t", p=128, k=KI_N
                    ),
                )
                cosc = cosT[:, c * 512:(c + 1) * 512]
                sinc = sinT[:, c * 512:(c + 1) * 512]
                for h in range(HPC):
                    acc = psp.tile([128, 512], F32, name="acc")
                    for ki in range(KI_N):
                        nc.tensor.matmul(
                            acc[:],
                            wt[:, h, ki, :],
                            xc[:, ki, :],
                            start=(ki == 0), stop=(ki == KI_N - 1),
                        )
                    qsb = sp.tile([128, 512], F32R, name="qsb")
                    nc.scalar.activation(qsb[:], acc[:], AF.Identity)
                    rot = prp.tile([128, 512], F32, name="rot")
                    nc.tensor.matmul(
                        rot[:], prot[:], qsb[:],
                        start=True, stop=True,
                    )
                    # rq = qsb*cos + rot*sin  (3 fused DVE ops)
                    sq = sp.tile([128, 512], F32R, name="sq")
                    nc.vector.scalar_tensor_tensor(
                        sq[:], rot[:], 1.0, sinc, ALU.mult, ALU.mult
                    )
                    cm = sp.tile([128, 512], F32R, name="cm")
                    nc.vector.scalar_tensor_tensor(
                        cm[:], qsb[:], 1.0, cosc, ALU.mult, ALU.mult
                    )
                    nc.vector.scalar_tensor_tensor(
                        dst[:, h, c * 512:(c + 1) * 512],
                        cm[:], 1.0, sq[:], ALU.mult, ALU.add,
                    )

    with tc.tile_pool(name="ropec", bufs=1) as rcp:
        cosT = rcp.tile([128, T], F32R, name="cosT")
        sinT = rcp.tile([128, T], F32R, name="sinT")
        nc.sync.dma_start(
            cosT[:], blob[COS_OFF:COS_OFF + 128 * T].rearrange(
                "(p t) -> p t", p=128
            ),
        )
        nc.sync.dma_start(
            sinT[:], blob[SIN_OFF:SIN_OFF + 128 * T].rearrange(
                "(p t) -> p t", p=128
            ),
        )
        qk_pass(0, qT)
        qk_pass(1, kT)

    # ---------------- v-pass: v_res[t_local, ktile, hd] ----------------
    vres_pool = stk.enter_context(tc.tile_pool(name="vres", bufs=1))
    v_res = vres_pool.tile([128, TPB, DL], F32R, name="v_res")
    with (
        tc.tile_pool(name="wv", bufs=1) as wvp,
        tc.tile_pool(name="xv", bufs=2) as xvp,
        tc.tile_pool(name="psv", bufs=2, space="PSUM") as psvp,
    ):
        wv = wvp.tile([128, KI_N, 512], F32R, name="wv")
        nc.sync.dma_start(
            wv[:],
            blob[WV_OFF:WV_OFF + 128 * KI_N * 512].rearrange(
                "(p k c) -> p k c", p=128, k=KI_N
            ),
        )
        for c in range(CH_N):
            xc = xvp.tile([128, KI_N, 512], F32R, name="xc")
            xoff = X4_OFF + c * 128 * KI_N * 512
            nc.sync.dma_start(
                xc[:],
                blob[xoff:xoff + 128 * KI_N * 512].rearrange(
                    "(p k t) -> p k t", p=128, k=KI_N
                ),
            )
            for tl in range(4):
                tt = c * 4 + tl
                acc = psvp.tile([128, 512], F32, name="acc")
                for ki in range(KI_N):
                    nc.tensor.matmul(
                        acc[:],
                        xc[:, ki, tl * 128:(tl + 1) * 128],
                        wv[:, ki, :],
                        start=(ki == 0), stop=(ki == KI_N - 1),
                    )
                nc.scalar.activation(v_res[:, tt, :], acc[:], AF.Identity)

    # ---------------- phase 2+3: attention + out-projection ----------------
    with (
        tc.tile_pool(name="p2", bufs=1) as p2,
        tc.tile_pool(name="p2w", bufs=3) as p2w,
        tc.tile_pool(name="p2o", bufs=1) as p2o,
        tc.tile_pool(name="stps", bufs=2, space="PSUM") as stps,
        tc.tile_pool(name="otps", bufs=2, space="PSUM") as otps,
        tc.tile_pool(name="smps", bufs=1, space="PSUM") as smps,
        tc.tile_pool(name="bcps", bufs=1, space="PSUM") as bcps,
        tc.tile_pool(name="p3ps", bufs=2, space="PSUM") as p3ps,
        tc.tile_pool(name="p3w", bufs=2) as p3w,
    ):
        wstep = p2.tile([128, 128], F32R, name="wstep")
        nc.sync.dma_start(
            wstep[:], blob[WSTEP_OFF:WSTEP_OFF + 128 * 128].rearrange(
                "(p j) -> p j", p=128
            ),
        )
        mskm = p2.tile([128, 4, 512], F32R, name="mskm")
        nc.sync.dma_start(
            mskm[:], blob[MSK_OFF:MSK_OFF + 128 * 4 * 512].rearrange(
                "(p r q) -> p r q", p=128, r=4
            ),
        )
        wout = p2.tile([128, HPC, D], F32R, name="wout")
        nc.sync.dma_start(
            wout[:],
            blob[WO_OFF:WO_OFF + 128 * HPC * D].rearrange(
                "(p h n) -> p h n", p=128, h=HPC
            ),
        )
        ones_r = wstep[0:1, 0:128]          # row j=0 of the step matrix
        ones_c = wstep[0:128, 127:128]      # col kp=127 (j<=127 always)

        outT_sb = p2o.tile([128, HPC, 512], F32R, name="outT")
        ou_sb = p2o.tile([128, HPC, 512], F32R, name="ou_sb")
        sums_sb = p2o.tile([1, HPC * 512], F32R, name="sums_sb")
        recip_sb = p2o.tile([1, HPC * 512], F32R, name="recip_sb")

        for qc in range(QC_N):
            n_ki = 4 * qc + 4
            for h in range(HPC):
                oT = otps.tile([128, 512], F32, name="oT")
                sums = smps.tile([1, 512], F32, name="sums")
                for ki in range(n_ki):
                    st = stps.tile([128, 512], F32, name="st")
                    diag = ki - 4 * qc
                    nc.tensor.matmul(
                        st[:],
                        kT[:, h, ki * 128:(ki + 1) * 128],
                        qT[:, h, qc * 512:(qc + 1) * 512],
                        start=True, stop=(diag < 0),
                    )
                    if diag >= 0:
                        nc.tensor.matmul(
                            st[:],
                            wstep[:],
                            mskm[:, diag, :],
                            start=False, stop=True,
                        )
                    pt = p2w.tile([128, 512], F32R, name="pt")
                    nc.scalar.activation(pt[:], st[:], AF.Exp, scale=SCALE)
                    nc.tensor.matmul(
                        sums[:], ones_c, pt[:],
                        start=(ki == 0), stop=(ki == n_ki - 1),
                    )
                    nc.tensor.matmul(
                        oT[:],
                        v_res[:, ki, h * 128:(h + 1) * 128],
                        pt[:],
                        start=(ki == 0), stop=(ki == n_ki - 1),
                    )
                nc.scalar.activation(
                    sums_sb[0:1, h * 512:(h + 1) * 512], sums[:], AF.Identity
                )
                nc.scalar.activation(ou_sb[:, h, :], oT[:], AF.Identity)
            nc.vector.reciprocal(recip_sb[:], sums_sb[:])
            for h2 in range(HPC):
                bc = bcps.tile([128, 512], F32, name="bc")
                nc.tensor.matmul(
                    bc[:], ones_r,
                    recip_sb[0:1, h2 * 512:(h2 + 1) * 512],
                    start=True, stop=True,
                )
                bc_sb = p2w.tile([128, 512], F32R, name="bc_sb")
                nc.scalar.activation(bc_sb[:], bc[:], AF.Identity)
                nc.vector.scalar_tensor_tensor(
                    outT_sb[:, h2, :], ou_sb[:, h2, :], 1.0, bc_sb[:],
                    ALU.mult, ALU.mult,
                )

            # ---- output projection for this qc's four t-tiles ----
            for tl in range(4):
                qt = 4 * qc + tl
                y_sb = p3w.tile([128, D], BF16, name="y_sb")
                for nch in range(NC_N):
                    y_ps = p3ps.tile([128, 512], F32, name="y_ps")
                    for h in range(HPC):
                        nc.tensor.matmul(
                            y_ps[:],
                            outT_sb[:, h, tl * 128:(tl + 1) * 128],
                            wout[:, h, nch * 512:(nch + 1) * 512],
                            start=(h == 0), stop=(h == HPC - 1),
                        )
                    nc.scalar.activation(
                        y_sb[:, nch * 512:(nch + 1) * 512], y_ps[:], AF.Identity
                    )
                eng = nc.sync if qt % 2 == 0 else nc.scalar
                eng.dma_start(y[qt * 128:(qt + 1) * 128, :], y_sb[:])


def build_program(reps=None, tiny_out=False):
    nc = bass.Bass(enable_partition_id=False)
    io = {}
    io["blob"] = nc.dram_tensor("blob", [BLOB_N], F32R, kind="ExternalInput")
    if tiny_out:
        io["y"] = nc.dram_tensor("y", [T, D], BF16)
        io["probe"] = nc.dram_tensor(
            "probe", [128, 512], BF16, kind="ExternalOutput"
        )
    else:
        io["y"] = nc.dram_tensor("y", [T, D], BF16, kind="ExternalOutput")

    from contextlib import ExitStack

    with tile.TileContext(nc) as tc:
        with nc.allow_low_precision(reason="float32r matmul pipeline"):
            with ExitStack() as stk:
                if reps is not None:
                    stk.enter_context(tc.For_i(0, reps, 1))
                _emit_body(nc, tc, io, stk)
                if tiny_out:
                    po = stk.enter_context(tc.tile_pool(name="po", bufs=1))
                    ot = po.tile([128, 512], BF16, name="ot")
                    nc.any.memset(ot[:], 2.0)
                    nc.sync.dma_start(io["probe"][:], ot[:])

    _split_multi_waits(nc)
    return nc


def host_inputs(x, w_qkv, w_out):
    """Build the 8 per-core input maps from the full problem inputs."""
    x = np.asarray(x, dtype=np.float32)
    w_qkv = np.asarray(w_qkv, dtype=np.float32)
    w_out = np.asarray(w_out, dtype=np.float32)

    # RoPE caches in [dim-partition, t] layout (match reference._rope_cache)
    inv_freq = 1.0 / (
        ROPE_THETA ** (np.arange(0, HD, 2, dtype=np.float32) / HD)
    )
    tpos = np.arange(T, dtype=np.float32)
    ang = tpos[None, :] * np.concatenate([inv_freq, inv_freq])[:, None]
    cosT = np.cos(ang).astype(np.float32)        # [128, T]
    sinT = np.sin(ang).astype(np.float32)        # [128, T] (unsigned)

    # rotate-half permutation as stationary lhsT:
    # out[j,t] = sum_d lhsT[d,j] * in[d,t];  rot[j] = -in[j+64] (j<64),
    # rot[j] = +in[j-64] (j>=64)
    prot = np.zeros((128, 128), np.float32)
    for j in range(64):
        prot[j + 64, j] = -1.0
        prot[j, j + 64] = 1.0

    # step matrix [j, kp] = 1 if j <= kp
    jj = np.arange(128)
    wstep = (jj[:, None] <= jj[None, :]).astype(np.float32)

    # shifted-delta mask matrices M_r [j, qf]: masked iff kp >= qf-128r+1
    qf = np.arange(512)[None, :]
    mskm = np.zeros((128, 4, 512), np.float32)
    for r in range(4):
        jstar = qf - 128 * r + 1                      # [1, 512]
        mskm[0, r, :] += NEG * (jstar[0] <= 0)
        for j in range(1, 128):
            mskm[j, r, :] = NEG * (jstar[0] == j)

    ki = np.arange(KI_N)

    in_maps = []
    for core in range(N_CORES):
        b = core // CPG
        g = core % CPG
        blob = np.empty(BLOB_N, np.float32)

        # X4: [chunk, p, ki, t] = x[b, c*512+t, ki*128+p]
        xb = x[b]                                    # [T, D]
        x4 = xb.reshape(CH_N, 512, KI_N, 128).transpose(0, 3, 2, 1)
        blob[X4_OFF:X4_OFF + x4.size] = np.ascontiguousarray(x4).reshape(-1)

        # WQ/WK: [p, h, ki, j] = w_qkv[ki*128+p, off + g*512 + h*128 + j]
        for which, woff, coloff in ((0, WQ_OFF, 0), (1, WK_OFF, D)):
            wcols = w_qkv[:, coloff + g * DL: coloff + (g + 1) * DL]
            wt = wcols.reshape(KI_N, 128, HPC, 128).transpose(1, 2, 0, 3)
            blob[woff:woff + wt.size] = np.ascontiguousarray(wt).reshape(-1)

        # WV: [p, ki, c] = w_qkv[ki*128+p, 2D + g*512 + c]
        wv = w_qkv[:, 2 * D + g * DL: 2 * D + (g + 1) * DL]
        wv = wv.reshape(KI_N, 128, DL).transpose(1, 0, 2)
        blob[WV_OFF:WV_OFF + wv.size] = np.ascontiguousarray(wv).reshape(-1)

        # WO: [p, h, n] = w_out[g*512 + h*128 + p, n]
        wo = w_out[g * DL:(g + 1) * DL, :].reshape(HPC, 128, D)
        wo = wo.transpose(1, 0, 2)
        blob[WO_OFF:WO_OFF + wo.size] = np.ascontiguousarray(wo).reshape(-1)

        blob[COS_OFF:COS_OFF + cosT.size] = cosT.reshape(-1)
        blob[SIN_OFF:SIN_OFF + sinT.size] = sinT.reshape(-1)
        blob[MSK_OFF:MSK_OFF + mskm.size] = mskm.reshape(-1)
        blob[WSTEP_OFF:WSTEP_OFF + wstep.size] = wstep.reshape(-1)
        blob[PROT_OFF:PROT_OFF + prot.size] = prot.reshape(-1)
        in_maps.append({"blob": blob})
    return in_maps


_NC_CACHE = {}


def kernel(x, w_qkv, w_out):
    if "nc" not in _NC_CACHE:
        _NC_CACHE["nc"] = build_program()
    nc = _NC_CACHE["nc"]
    in_maps = host_inputs(x, w_qkv, w_out)
    res = run_bass_kernel_spmd(nc, in_maps, list(range(N_CORES)))
    y = np.zeros((B, T, D), dtype=np.float64)
    for c in range(N_CORES):
        y[c // CPG] += res.results[c]["y"].astype(np.float64)
    return y.astype(np.float32)
